# revision 2
# baseline (speedup 1.0000x reference)
"""Trainium2 Bass kernel for nn_CELLI_29850022707545 (GNN message passing).

Strategy (graph/node-parallel per the sharding hint): host shards edges by
sender-node range across the 8 cores (CSR-style graph partitioning).  Each
core owns a contiguous node range and ALL edges pointing into it, so the
segment-sum is core-local (no collectives).  On device, per-core:

  pass 1: chi = mlp(x) per edge (feature-major matmuls), env(|v|) per edge
          (edge-major grid), chi*env -> per-partition prefix sums (cumsum).
  node:   per-run (per-node) sums extracted with forward/backward hold-scans
          over the edge grid; charges per edge derived in-place.
  pass 2: main MLP with the charges row and the (host-prepared, parameter-
          only) charge_embed[species[senders]] rows folded into layer 1;
          out = env * h3.

Edge grid: [128 partitions x C columns], partition p holds edges
[p*C, (p+1)*C) of the device order; host pads each partition to start on a
node boundary so every node's edges live inside one partition row.
"""

import numpy as np
import ml_dtypes

BF16 = ml_dtypes.bfloat16

N_NODES = 50000
F_IN = 32
EMB = 16
HID = 32
P = 128
C = 1600              # grid columns per partition
EP = P * C            # padded edges per core (204800)
GCH = 64              # grid-chunk columns
NGCH = C // GCH       # 25 grid chunks
ROWN = P * GCH        # 8192 edges per grid chunk
NMM = ROWN // 512     # 16 psum chunks per grid chunk
NCORES = 8
ENV_P = 6


def _host_prep(vectors, x, senders, species, W_chi1, W_chi2, radius, hardness,
               charge_embed, scale, shift, W_w, W_x1, W_x2, W_x3):
    E = senders.shape[0]
    senders = np.asarray(senders).astype(np.int64)
    species = np.asarray(species).astype(np.int64)

    # ---- parameter folding (compile-time constant work) ----
    s48 = 1.0 / np.sqrt(48.0)
    s17 = 1.0 / np.sqrt(17.0)
    Wc1 = (np.asarray(W_chi1) / np.sqrt(32.0)).astype(np.float32)      # [32,16]
    Wc2 = (np.asarray(W_chi2) / np.sqrt(16.0)).astype(np.float32)      # [16,1]
    W1a = (np.asarray(W_x1)[:32] * s48).astype(np.float32)             # [32,32]
    W1b = (np.asarray(W_x1)[32:] * s48).astype(np.float32)             # [16,32]
    Wce = (np.asarray(W_w)[1:] * s17 @ W1b).astype(np.float32)         # [16,32]
    vch = (np.asarray(W_w)[0] * s17 @ W1b).astype(np.float32)          # [32]
    W2 = (np.asarray(W_x2) / np.sqrt(32.0)).astype(np.float32)
    W3 = (np.asarray(W_x3) / np.sqrt(32.0)).astype(np.float32)

    def softplus(z):
        return np.log1p(np.exp(-np.abs(z))) + np.maximum(z, 0.0)

    hard = softplus(np.asarray(hardness, np.float64))                  # [100]
    gam = softplus(np.asarray(radius, np.float64)) / np.log(2.0)       # [100]
    hardInv_t = (1.0 / hard).astype(np.float32)
    c2_t = (-0.5 / hard + 0.01 * gam / hard**2).astype(np.float32)

    # ---- shard edges by sender range (stable sort = CSR build) ----
    perm = np.argsort(senders, kind="stable")
    ssend = senders[perm]
    # balanced cut points on edges, snapped to node boundaries
    cuts = [0]
    for i in range(1, NCORES):
        pos = (E * i) // NCORES
        n = ssend[min(pos, E - 1)]
        pos = np.searchsorted(ssend, n, side="left")
        cuts.append(int(pos))
    cuts.append(E)
    node_cuts = [0] + [int(ssend[c]) if c < E else N_NODES for c in cuts[1:-1]] + [N_NODES]

    shards = []
    for i in range(NCORES):
        lo, hi = cuts[i], cuts[i + 1]
        n0, n1 = node_cuts[i], node_cuts[i + 1]
        eidx = perm[lo:hi]                         # original edge ids, sorted by sender
        snd = ssend[lo:hi]
        # per-node degree over the full owned range (zeros included)
        deg = np.bincount(snd - n0, minlength=n1 - n0)
        deg = np.maximum(deg, 1)                   # empty node -> one pad edge
        # pack whole nodes into partitions of capacity C
        node_part = np.zeros(n1 - n0, np.int64)
        part_fill = np.zeros(P, np.int64)
        pcur = 0
        for nl in range(n1 - n0):
            d = deg[nl]
            if part_fill[pcur] + d > C:
                pcur += 1
                assert pcur < P, "partition overflow; raise C"
            node_part[nl] = pcur
            part_fill[pcur] += d
        # device slot for each node's edge block
        node_off = np.zeros(n1 - n0, np.int64)
        fill2 = np.zeros(P, np.int64)
        for nl in range(n1 - n0):
            pp = node_part[nl]
            node_off[nl] = pp * C + fill2[pp]
            fill2[pp] += deg[nl]
        # per-edge device position
        within = np.zeros(hi - lo, np.int64)
        if hi > lo:
            first = np.r_[True, snd[1:] != snd[:-1]]
            runid = np.cumsum(first) - 1
            starts = np.flatnonzero(first)
            within = np.arange(hi - lo) - starts[runid]
        dpos = node_off[snd - n0] + within          # device slot of each real edge
        end_pos = node_off + deg - 1                # per-node end slot
        shards.append(dict(eidx=eidx, snd=snd, n0=n0, n1=n1, dpos=dpos,
                           deg=deg, end_pos=end_pos))

    consts = dict(Wc1=Wc1, Wc2=Wc2, W1a=W1a, Wce=Wce, vch=vch, W2=W2, W3=W3,
                  hardInv_t=hardInv_t, c2_t=c2_t,
                  scale=float(np.asarray(scale)), shift=float(np.asarray(shift)),
                  ce_table=np.asarray(charge_embed, np.float32))
    return shards, consts


def _chunk_order_cols():
    # device-order j within a grid chunk: j = p*GCH + cc -> grid (p, 64k+cc)
    j = np.arange(ROWN)
    return j // GCH, j % GCH       # (partition, col-within-chunk)


def _grid_to_stream(a_grid):
    """[P, C] -> [EP] in stream order (chunk-major, then p*64+cc)."""
    out = np.empty(EP, a_grid.dtype)
    jp, jc = _chunk_order_cols()
    for k in range(NGCH):
        out[k * ROWN:(k + 1) * ROWN] = a_grid[jp, k * GCH + jc]
    return out


def _stream_pos(dpos):
    """device grid slot (p*C + c) -> stream position."""
    p = dpos // C
    c = dpos % C
    k = c // GCH
    cc = c % GCH
    return k * ROWN + p * GCH + cc


def _build_core_inputs(shard, consts, vectors, x, species):
    """Build all per-core device arrays (host layout only)."""
    eidx, snd, n0 = shard["eidx"], shard["snd"], shard["n0"]
    dpos = shard["dpos"]
    spos = _stream_pos(dpos)

    sp_e = species[snd]                                  # per-edge species
    # stream-ordered inputs, padding defaults
    xT = np.zeros((F_IN, EP), BF16)
    ceT = np.zeros((EMB, EP), BF16)
    v0 = np.full((P, C), 2.0, np.float32)                # pad length 2 -> env 0
    v1 = np.zeros((P, C), np.float32)
    v2 = np.zeros((P, C), np.float32)
    hardInv = np.ones((P, C), np.float32)
    c2 = np.zeros((P, C), BF16)
    isend = np.zeros((P, C), np.float32)

    xs = np.asarray(x)[eidx].astype(np.float32)          # [e,32]
    xT[:, spos] = xs.T.astype(BF16)
    ceT[:, spos] = consts["ce_table"][sp_e].T.astype(BF16)
    vv = np.asarray(vectors)[eidx].astype(np.float32)
    gp, gc = dpos // C, dpos % C
    v0[gp, gc] = vv[:, 0]
    v1[gp, gc] = vv[:, 1]
    v2[gp, gc] = vv[:, 2]
    hardInv[gp, gc] = consts["hardInv_t"][sp_e]

    # per-node (incl. empty/pad nodes): end markers + species params at ends
    npn = shard["n1"] - n0
    ep = shard["end_pos"]
    epg, epc = ep // C, ep % C
    isend[epg, epc] = 1.0
    sp_n = species[n0:shard["n1"]]
    c2[epg, epc] = consts["c2_t"][sp_n].astype(BF16)
    hardInv[epg, epc] = consts["hardInv_t"][sp_n]        # ends definitely right
    # hardInv must be right for EVERY edge of the run (charges row):
    hardInv[gp, gc] = consts["hardInv_t"][sp_e]

    return dict(xT=xT, ceT=ceT, v0=v0, v1=v1, v2=v2,
                hardInv=hardInv, c2=c2, isend=isend)


def _golden_core(core_in, consts):
    """Numpy model of the device kernel for one core (device order)."""
    xT = core_in["xT"].astype(np.float32)
    ceT = core_in["ceT"].astype(np.float32)

    def silu(z):
        return z / (1.0 + np.exp(-z))

    # pass 1: chi per edge (stream order) -> grid
    h1 = silu(xT.T @ consts["Wc1"])
    chi_s = (h1 @ consts["Wc2"])[:, 0]                   # [EP] stream order
    chi = np.zeros((P, C), np.float32)
    jp, jc = _chunk_order_cols()
    for k in range(NGCH):
        chi[jp, k * GCH + jc] = chi_s[k * ROWN:(k + 1) * ROWN]

    r2 = core_in["v0"]**2 + core_in["v1"]**2 + core_in["v2"]**2
    r = np.sqrt(r2)
    a = -(ENV_P + 1) * (ENV_P + 2) / 2.0
    b = float(ENV_P * (ENV_P + 2))
    cc = -ENV_P * (ENV_P + 1) / 2.0
    r6 = r2 * r2 * r2
    env = 1.0 + r6 * (a + b * r + cc * r2)
    env = np.where(r < 1.0, env, 0.0).astype(np.float32)

    chienv = chi * env
    cum = np.cumsum(chienv, axis=1, dtype=np.float32)    # per-partition prefix

    isend = core_in["isend"]
    isstart = np.zeros_like(isend)
    isstart[:, 0] = 1.0
    isstart[:, 1:] = isend[:, :-1]

    # fwd hold of cum[start-1] (0 at partition start)
    cm1 = np.zeros_like(cum)
    cm1[:, 1:] = cum[:, :-1]
    cols = np.arange(C)[None, :]
    ffi = np.maximum.accumulate(np.where(isstart > 0, cols, 0), axis=1)
    Ch = np.take_along_axis(cm1, ffi, axis=1)
    U = cum - Ch                                          # run partial sums
    # bwd hold of U at ends -> per-edge full run sum
    bfi = (C - 1) - np.maximum.accumulate(
        np.where(isend[:, ::-1] > 0, cols, 0), axis=1)[:, ::-1]
    acc = np.take_along_axis(U, bfi, axis=1)

    chis = acc * consts["scale"] + consts["shift"]
    charges = -chis * core_in["hardInv"]
    pot = float(np.sum(isend * chis * chis * core_in["c2"].astype(np.float32)))

    # pass 2
    ch_s = _grid_to_stream(charges)
    env_s = _grid_to_stream(env)
    rhs2 = np.concatenate([ceT, ch_s[None, :]], axis=0)  # [17, EP]
    L1w = np.concatenate([consts["Wce"], consts["vch"][None, :]], axis=0)
    h1m = silu(xT.T @ consts["W1a"] + rhs2.T @ L1w)
    h2 = silu(h1m @ consts["W2"])
    h3 = h2 @ consts["W3"]
    outT = (h3 * env_s[:, None]).T                        # [32, EP] stream order
    return dict(outT=outT, charges=charges, pot=pot, env=env, chi=chi,
                cum=cum, U=U, acc=acc)


def host_reference_check():
    """Validate host prep + golden model against reference math (test helper)."""
    pass


def kernel(**inputs):
    from kernel_device import run_device  # device path (separate for testing)
    return _kernel_impl(inputs, run_device)


def _kernel_impl(inputs, run_device):
    vectors = np.asarray(inputs["vectors"], np.float32)
    x = np.asarray(inputs["x"], np.float32)
    V = inputs["V"]
    senders = np.asarray(inputs["senders"]).astype(np.int64)
    species = np.asarray(inputs["species"]).astype(np.int64)

    shards, consts = _host_prep(
        vectors, x, senders, species, inputs["W_chi1"], inputs["W_chi2"],
        inputs["radius"], inputs["hardness"], inputs["charge_embed"],
        inputs["scale"], inputs["shift"], inputs["W_w"], inputs["W_x1"],
        inputs["W_x2"], inputs["W_x3"])

    core_ins = [_build_core_inputs(sh, consts, vectors, x, species)
                for sh in shards]
    outs = run_device(core_ins, consts)   # list of dicts: outT [32,EP] f32-ish, charges grid, pot

    E = senders.shape[0]
    out = np.empty((E, F_IN), np.float32)
    charges_full = np.empty(N_NODES, np.float32)
    pot = np.float32(0.0)
    for i, sh in enumerate(shards):
        o = outs[i]
        spos = _stream_pos(sh["dpos"])
        out[sh["eidx"]] = np.asarray(o["outT"], np.float32).T[spos]
        chg = np.asarray(o["charges"], np.float32).reshape(P * C)
        charges_full[sh["n0"]:sh["n1"]] = chg[sh["end_pos"]]
        pot = pot + np.float32(o["pot"])
    return out, np.asarray(V), (charges_full, np.float32(pot))
